# revision 1
# baseline (speedup 1.0000x reference)
"""TRN2 Bass kernel for nn_DEAM_5076651343977 (dense_transformer).

Computation (per sample):
    d  = avg_pool8(diff)                      [C, 32, 32] -> [C, N=1024]
    q  = Wq d + bq ; k = Wk d + bk
    E[n,m] = sum_c q[c,n] k[c,m] * C^-0.5
    attn = softmax_m(E)
    v  = Wv avg_pool8(x) + bv
    out_small[c,n] = sum_m v[c,m] attn[n,m]
    out = repeat8(out_small) + x

Sharding: pure data parallel, one sample per NeuronCore (B=8 over 8 cores).

Per-core layout trick: partitions p = s*64 + c with s = hp%2 (h-block parity),
free = hpp*2048 + r*256 + w  (h = (2*hpp+s)*8 + r, w = wp*8 + i).
x stays resident in SBUF in this layout; pooling is one tensor_reduce(XY)
per 2048-wide block; the final upsample+residual is one tensor_tensor add
per block with a zero-stride broadcast AP on the small operand, written
in place over x and DMA'd out.

The avg-pool 1/64 and conv biases are folded into augmented weights
(K=65 with a ones row appended to the pooled activations).
Softmax max-subtraction is skipped: |0.125*E| is O(1) for these inputs
(q,k come from 8x8-averaged unit-variance data), far from fp32 exp range.
The softmax denominator comes for free as a 65th output row of the
out_small matmul (ones column appended to v^T).
"""
import numpy as np

import concourse.bass as bass
import concourse.mybir as mybir
from concourse import bacc
from concourse.tile import TileContext
from concourse.bass_utils import run_bass_kernel_spmd

f32 = mybir.dt.float32
fATT = mybir.dt.float32r  # rounded fp32: 4x faster PE, ~1.5e-4 rounding

B, C, H, W = 8, 64, 256, 256
DS = 8
HW = H * W            # 65536
NB = 16               # h-pair blocks per sample
BLK = 2048            # free elems per block per partition (8 rows x 256)

_cache = {}


def _xpack_dma(nc, dst, dram, hpp, store=False):
    """Move block hpp between DRAM x[c, h, w] and the (s,c)-packed SBUF tile
    (partition p = s*64+c, free f = r*256 + w, h = (2*hpp+s)*8 + r).

    Two DMAs (one per s-half): a 2-level partition walk in a single DMA AP
    runs at ~60 GB/s on HWDGE; single-level strides hit ~380 GB/s.
    """
    for s in range(2):
        ap = bass.AP(dram, hpp * 2 * BLK + s * BLK, [[HW, C], [1, BLK]])
        half = dst[s * 64:(s + 1) * 64, :]
        if store:
            nc.sync.dma_start(ap, half)
        else:
            nc.sync.dma_start(half, ap)


def _emit(nc, tc, pools, drams):
    big, stream, small, attnp, psA, psE, psO = pools
    x_d, diff_d, wq_d, wk_d, wv_d, out_d = drams
    RED = mybir.AluOpType.add
    XY = mybir.AxisListType.XY
    if True:
        if True:
            wq = small.tile([65, 64], f32, name="wq_sb")
            wk = small.tile([65, 64], f32, name="wk_sb")
            wv = small.tile([65, 64], f32, name="wv_sb")
            nc.gpsimd.dma_start(wq, wq_d[:, :])
            nc.gpsimd.dma_start(wk, wk_d[:, :])
            nc.gpsimd.dma_start(wv, wv_d[:, :])

            x_sb = big.tile([128, NB * BLK], f32, name="x_sb")
            pooled_x = small.tile([128, 512], f32, name="pooled_x")
            pooled_f = small.tile([128, 512], f32, name="pooled_f")

            d_aug = small.tile([65, 1024], f32, name="d_aug")
            px_aug = small.tile([65, 1024], f32, name="px_aug")
            nc.vector.memset(d_aug[64:65, :], 1.0)
            nc.vector.memset(px_aug[64:65, :], 1.0)
            vT = small.tile([128, 8 * 65], fATT, name="vT")
            nc.vector.memset(vT[:, :].bitcast(f32), 1.0)
            q_sb = small.tile([64, 1024], fATT, name="q_sb")
            k_sb = small.tile([64, 1024], fATT, name="k_sb")
            out_ps = psO.tile([65, 1024], f32, name="out_ps")

            # ---- phase 1a: stream diff (pool+discard), then q,k ----
            for hpp in range(NB):
                db = stream.tile([128, BLK], f32, name="db", tag="db")
                _xpack_dma(nc, db, diff_d, hpp)
                nc.vector.tensor_reduce(
                    pooled_f[:, hpp * 32:(hpp + 1) * 32],
                    db.rearrange("p (r wp i) -> p wp r i", r=8, wp=32, i=8),
                    axis=XY, op=RED)
            for s in range(2):
                # dest free index = hpp*64 + s*32 + wp  (n = hp*32+wp, hp=2*hpp+s)
                a0 = d_aug[0:64, :]
                dst = bass.AP(a0.tensor, a0.offset + s * 32,
                              [list(a0.ap[0]), [64, 16], [1, 32]])
                nc.gpsimd.dma_start(dst, pooled_f[s * 64:(s + 1) * 64, :])
            for (w_t, dst) in ((wq, q_sb), (wk, k_sb)):
                ps = psA.tile([64, 1024], f32, name="qk_ps", tag="psa")
                for ch in range(2):
                    nc.tensor.matmul(ps[:, ch * 512:(ch + 1) * 512], w_t[:, :],
                                     d_aug[:, ch * 512:(ch + 1) * 512],
                                     start=True, stop=True)
                nc.scalar.copy(dst[:, :], ps[:, :])

            # ---- phase 1b: stream x; attention paced per m-tile t ----
            # m-tile t needs only x blocks 2t, 2t+1 (via pooled_x -> v^T), so
            # ET/exp/out-accumulation retire alongside the x stream and the
            # output is finished right after the last x block lands.
            for t in range(8):
                for j in range(2):
                    hpp = 2 * t + j
                    xs = x_sb[:, hpp * BLK:(hpp + 1) * BLK]
                    _xpack_dma(nc, xs, x_d, hpp)
                    nc.vector.tensor_reduce(
                        pooled_x[:, hpp * 32:(hpp + 1) * 32],
                        xs.rearrange("p (r wp i) -> p wp r i", r=8, wp=32, i=8),
                        axis=XY, op=RED)
                    for s in range(2):
                        a0 = px_aug[0:64, :]
                        dst = bass.AP(a0.tensor, a0.offset + hpp * 64 + s * 32,
                                      [[a0.ap[0][0], 64], [1, 32]])
                        nc.gpsimd.dma_start(
                            dst, pooled_x[s * 64:(s + 1) * 64,
                                          hpp * 32:(hpp + 1) * 32])
                vps = psA.tile([128, 64], f32, name="vps", tag="psa")
                nc.tensor.matmul(vps[:, :], px_aug[:, t * 128:(t + 1) * 128],
                                 wv[:, :], start=True, stop=True)
                nc.scalar.copy(vT[:, t * 65:t * 65 + 64], vps[:, :])
                et = psE.tile([128, 1024], f32, name="et", tag="et")
                for ch in range(2):
                    nc.tensor.matmul(et[:, ch * 512:(ch + 1) * 512],
                                     k_sb[:, t * 128:(t + 1) * 128],
                                     q_sb[:, ch * 512:(ch + 1) * 512],
                                     start=True, stop=True)
                at = attnp.tile([128, 1024], fATT, name="at", tag="at")
                nc.scalar.activation(at[:, :], et[:, :],
                                     mybir.ActivationFunctionType.Exp, scale=0.125)
                for ch in range(2):
                    nc.tensor.matmul(out_ps[:, ch * 512:(ch + 1) * 512],
                                     vT[:, t * 65:(t + 1) * 65],
                                     at[:, ch * 512:(ch + 1) * 512],
                                     start=(t == 0), stop=(t == 7))

            # ---- phase 4: normalize by softmax sums (row 64 of out_ps) ----
            den_sb = small.tile([1, 1024], fATT, name="den_sb")
            nc.scalar.copy(den_sb[:, :], out_ps[64:65, :])
            ones1 = small.tile([1, 64], fATT, name="ones1")
            nc.vector.memset(ones1[:, :].bitcast(f32), 1.0)
            rb_ps = psA.tile([64, 1024], f32, name="rb_ps", tag="psa")
            for ch in range(2):
                nc.tensor.matmul(rb_ps[:, ch * 512:(ch + 1) * 512], ones1[:, :],
                                 den_sb[:, ch * 512:(ch + 1) * 512],
                                 start=True, stop=True)
            rb_sb = small.tile([64, 1024], f32, name="rb_sb")
            nc.vector.reciprocal(rb_sb[:, :], rb_ps[:, :])
            osn = small.tile([64, 1024], f32, name="osn")
            nc.vector.tensor_tensor(osn[:, :], out_ps[0:64, :], rb_sb[:, :],
                                    mybir.AluOpType.mult)

            # ---- phase 5: pack os -> (s,c) layout, upsample+add, store ----
            os2 = small.tile([128, 512], f32, name="os2")
            for s in range(2):
                src = bass.AP(osn.tensor, osn.offset + s * 32,
                              [list(osn.ap[0]), [64, 16], [1, 32]])
                nc.sync.dma_start(os2[s * 64:(s + 1) * 64, :], src)

            for hpp in range(NB):
                xs = x_sb[:, hpp * BLK:(hpp + 1) * BLK]
                ob = stream.tile([128, BLK], f32, name="ob", tag="db")
                up = bass.AP(os2.tensor, os2.offset + hpp * 32,
                             [list(os2.ap[0]), [0, 8], [1, 32], [0, 8]])
                nc.vector.tensor_tensor(
                    ob.rearrange("p (r wp i) -> p r wp i", r=8, wp=32, i=8),
                    xs.rearrange("p (r wp i) -> p r wp i", r=8, wp=32, i=8),
                    up, mybir.AluOpType.add)
                _xpack_dma(nc, ob, out_d, hpp, store=True)


def _build(dup=1):
    nc = bacc.Bacc("TRN2", target_bir_lowering=False, debug=False, num_devices=8)

    x_d = nc.dram_tensor("x", [C, HW], f32, kind="ExternalInput")
    diff_d = nc.dram_tensor("diff", [C, HW], f32, kind="ExternalInput")
    wq_d = nc.dram_tensor("wq", [65, 64], f32, kind="ExternalInput")
    wk_d = nc.dram_tensor("wk", [65, 64], f32, kind="ExternalInput")
    wv_d = nc.dram_tensor("wv", [65, 64], f32, kind="ExternalInput")
    out_d = nc.dram_tensor("out", [C, HW], f32, kind="ExternalOutput")
    drams = (x_d, diff_d, wq_d, wk_d, wv_d, out_d)

    with TileContext(nc) as tc:
        with tc.tile_pool(name="big", bufs=1) as big, \
             tc.tile_pool(name="stream", bufs=4) as stream, \
             tc.tile_pool(name="small", bufs=1) as small, \
             tc.tile_pool(name="attn", bufs=2) as attnp, \
             tc.tile_pool(name="psA", bufs=1, space="PSUM") as psA, \
             tc.tile_pool(name="psE", bufs=2, space="PSUM") as psE, \
             tc.tile_pool(name="psO", bufs=1, space="PSUM") as psO:
            pools = (big, stream, small, attnp, psA, psE, psO)
            for rep in range(dup):
                if rep:
                    tc.strict_bb_all_engine_barrier()
                _emit(nc, tc, pools, drams)

    nc.compile()
    return nc


def make_in_maps(inputs):
    x = np.ascontiguousarray(np.asarray(inputs["x"], dtype=np.float32))
    diff = np.ascontiguousarray(np.asarray(inputs["diff"], dtype=np.float32))
    # fold avg-pool 1/64 into the weights; append bias row (K=65 aug trick)
    inv = 1.0 / (DS * DS)
    wq_aug = np.concatenate(
        [np.asarray(inputs["Wq"]).T * inv, np.asarray(inputs["bq"])[None, :]], 0)
    wk_aug = np.concatenate(
        [np.asarray(inputs["Wk"]).T * inv, np.asarray(inputs["bk"])[None, :]], 0)
    wv_aug = np.concatenate(
        [np.asarray(inputs["Wv"]).T * inv, np.asarray(inputs["bv"])[None, :]], 0)
    wq_aug = np.ascontiguousarray(wq_aug, dtype=np.float32)
    wk_aug = np.ascontiguousarray(wk_aug, dtype=np.float32)
    wv_aug = np.ascontiguousarray(wv_aug, dtype=np.float32)
    return [
        {
            "x": x[b].reshape(C, HW),
            "diff": diff[b].reshape(C, HW),
            "wq": wq_aug, "wk": wk_aug, "wv": wv_aug,
        }
        for b in range(B)
    ]


def kernel(x, diff, Wq, bq, Wk, bk, Wv, bv):
    if "nc" not in _cache:
        _cache["nc"] = _build()
    nc = _cache["nc"]

    in_maps = make_in_maps(dict(x=x, diff=diff, Wq=Wq, bq=bq, Wk=Wk, bk=bk,
                                Wv=Wv, bv=bv))
    res = run_bass_kernel_spmd(nc, in_maps, list(range(B)))
    out = np.stack([res.results[b]["out"].reshape(C, H, W) for b in range(B)])
    return out.astype(np.float32)


if __name__ == "__main__":
    rng = np.random.default_rng(0)
    xs = rng.standard_normal((B, C, H, W), dtype=np.float32)
    ds = rng.standard_normal((B, C, H, W), dtype=np.float32)
    sc = 1.0 / np.sqrt(C)
    args = dict(
        x=xs, diff=ds,
        Wq=rng.standard_normal((C, C), dtype=np.float32) * sc,
        bq=rng.standard_normal(C, dtype=np.float32) * 0.01,
        Wk=rng.standard_normal((C, C), dtype=np.float32) * sc,
        bk=rng.standard_normal(C, dtype=np.float32) * 0.01,
        Wv=rng.standard_normal((C, C), dtype=np.float32) * sc,
        bv=rng.standard_normal(C, dtype=np.float32) * 0.01,
    )
    out = kernel(**args)
    print("kernel ran, out shape", out.shape, out.dtype)



# revision 3
# speedup vs baseline: 1.6037x; 1.6037x over previous
"""TRN2 Bass kernel for nn_DEAM_5076651343977 (dense_transformer).

Computation (per sample):
    d  = avg_pool8(diff)                      [C, 32, 32] -> [C, N=1024]
    q  = Wq d + bq ; k = Wk d + bk
    E[n,m] = sum_c q[c,n] k[c,m] * C^-0.5
    attn = softmax_m(E)
    v  = Wv avg_pool8(x) + bv
    out_small[c,n] = sum_m v[c,m] attn[n,m]
    out = repeat8(out_small) + x

Sharding: pure data parallel, one sample per NeuronCore (B=8 over 8 cores).

The kernel is HBM-bound, so I/O dtypes are narrowed (gate is rel<2e-2;
measured 7e-4 end-to-end with this scheme): x and out travel as fp16
(8MB each per core), diff as fp8-e4m3 (4MB) cast to fp16 during the
SWDGE load. 20MB/core total vs 48MB for fp32.

Per-core layout: partitions p = s*64 + c with s = hp%2 (h-block parity),
free = hpp*2048 + r*256 + wp*8 + i  (h = (2*hpp+s)*8 + r, w = wp*8 + i).
Attention tokens use the s-major order n' = s*512 + hpp*32 + wp so every
layout pack is a contiguous [64, X] SBUF->SBUF DMA (softmax is invariant
to a consistent token permutation; the upsample AP inverts it for free).

Big DMAs move 4-block groups (1MB each at fp16) in 2 per-s-half
transfers (single-level partition walk). Pooling is a two-stage DVE
reduce (dense 8:1 innermost, then strided 8:1 over rows). The avg-pool
1/64 and conv biases fold into augmented weights (K=65 ones-row trick);
the softmax denominator falls out of the out-matmul as row 64 (ones
column in v^T). Softmax max-subtraction is skipped: |0.125*E| is O(1)
for 8x8-averaged unit-variance inputs, far from fp32 exp range.
Layout packs ride the ACT HWDGE ring so they never head-of-line block
the x-load/store stream on the SP ring.
"""
import numpy as np

import concourse.bass as bass
import concourse.mybir as mybir
from concourse import bacc
from concourse.tile import TileContext
from concourse.bass_utils import run_bass_kernel_spmd

f32 = mybir.dt.float32
f16 = mybir.dt.float16
f8 = mybir.dt.float8e4
fATT = mybir.dt.float32r  # rounded fp32: 4x faster PE, ~1.5e-4 rounding

B, C, H, W = 8, 64, 256, 256
DS = 8
HW = H * W            # 65536
NB = 16               # h-pair blocks per sample
BLK = 2048            # free elems per block per partition (8 rows x 256)
G = 4                 # blocks per DMA/pool group
NG = NB // G          # 4 groups
GBLK = G * BLK        # 8192

# True: load diff as fp8 into SBUF and let DVE pool it directly.
# False: SWDGE cast-DMA fp8->fp16 during the load; DVE pools fp16.
DIFF_FP8_DVE = False

_cache = {}


def _group_ap(dram, g, s):
    """DRAM AP for the s-half of 4-block group g: per channel c, G runs
    of BLK at stride 2*BLK, base g*2*GBLK + s*BLK."""
    return bass.AP(dram, g * 2 * GBLK + s * BLK,
                   [[HW, C], [2 * BLK, G], [1, BLK]])


def _pool(nc, dst128, src, stg):
    """dst (f32, [128, G*32] as (blk,wp)) = 8x8 pool sums of one 4-block
    group src [128, GBLK] laid (blk, r, wp, i); two-stage reduce."""
    RED = mybir.AluOpType.add
    Xax = mybir.AxisListType.X
    nc.vector.tensor_reduce(
        stg[:, :], src.rearrange("p (f i) -> p f i", i=DS),
        axis=Xax, op=RED)
    nc.vector.tensor_reduce(
        dst128.rearrange("p (blk wp) -> p blk wp", blk=G),
        stg.rearrange("p (blk r wp) -> p blk wp r", blk=G, r=DS, wp=32),
        axis=Xax, op=RED)


def _emit(nc, tc, pools, drams):
    big, dstream, obuf, small, attnp, psA, psE, psO = pools
    x_d, diff_d, wq_d, wk_d, wv_d, out_d = drams

    wq = small.tile([65, 64], f32, name="wq_sb")
    wk = small.tile([65, 64], f32, name="wk_sb")
    wv = small.tile([65, 64], f32, name="wv_sb")
    nc.gpsimd.dma_start(wq, wq_d[:, :])
    nc.gpsimd.dma_start(wk, wk_d[:, :])
    nc.gpsimd.dma_start(wv, wv_d[:, :])

    x_sb = big.tile([128, NB * BLK], f16, name="x_sb")
    pooled_x = small.tile([128, 512], f32, name="pooled_x")
    pooled_f = small.tile([128, 512], f32, name="pooled_f")
    stg_f = small.tile([128, 1024], f32, name="stg_f")
    stg_x = small.tile([128, 1024], f32, name="stg_x")

    d_aug = small.tile([65, 1024], f32, name="d_aug")
    px_aug = small.tile([65, 1024], f32, name="px_aug")
    nc.vector.memset(d_aug[64:65, :], 1.0)
    nc.vector.memset(px_aug[64:65, :], 1.0)
    vT = small.tile([128, 8 * 65], fATT, name="vT")
    nc.vector.memset(vT[:, :].bitcast(f32), 1.0)
    q_sb = small.tile([64, 1024], fATT, name="q_sb")
    k_sb = small.tile([64, 1024], fATT, name="k_sb")
    out_ps = psO.tile([65, 1024], f32, name="out_ps")

    # ---- phase 1: stream diff by group (pool+discard), then q,k ----
    for g in range(NG):
        db = dstream.tile([128, GBLK], f8 if DIFF_FP8_DVE else f16,
                          name="db", tag="db")
        for s in range(2):
            half = db[s * 64:(s + 1) * 64, :]
            if DIFF_FP8_DVE:
                nc.sync.dma_start(half, _group_ap(diff_d, g, s))
            else:
                nc.gpsimd.dma_start(half, _group_ap(diff_d, g, s))
        _pool(nc, pooled_f[:, g * 128:(g + 1) * 128], db[:, :], stg_f)
    for s in range(2):
        nc.scalar.dma_start(d_aug[0:64, s * 512:(s + 1) * 512],
                            pooled_f[s * 64:(s + 1) * 64, :])
    for (w_t, dst) in ((wq, q_sb), (wk, k_sb)):
        ps = psA.tile([64, 1024], f32, name="qk_ps", tag="psa")
        for ch in range(2):
            nc.tensor.matmul(ps[:, ch * 512:(ch + 1) * 512], w_t[:, :],
                             d_aug[:, ch * 512:(ch + 1) * 512],
                             start=True, stop=True)
        nc.scalar.copy(dst[:, :], ps[:, :])

    # ---- phase 2: stream x by group; attention tiles (g, g+4) ----
    for g in range(NG):
        xs = x_sb[:, g * GBLK:(g + 1) * GBLK]
        for s in range(2):
            nc.sync.dma_start(xs[s * 64:(s + 1) * 64, :],
                              _group_ap(x_d, g, s))
        _pool(nc, pooled_x[:, g * 128:(g + 1) * 128], xs, stg_x)
        for s in range(2):
            nc.scalar.dma_start(
                px_aug[0:64, s * 512 + g * 128:s * 512 + (g + 1) * 128],
                pooled_x[s * 64:(s + 1) * 64, g * 128:(g + 1) * 128])
        for t in (g, g + 4):
            vps = psA.tile([128, 64], f32, name="vps", tag="psa")
            nc.tensor.matmul(vps[:, :], px_aug[:, t * 128:(t + 1) * 128],
                             wv[:, :], start=True, stop=True)
            nc.scalar.copy(vT[:, t * 65:t * 65 + 64], vps[:, :])
            et = psE.tile([128, 1024], f32, name="et", tag="et")
            for ch in range(2):
                nc.tensor.matmul(et[:, ch * 512:(ch + 1) * 512],
                                 k_sb[:, t * 128:(t + 1) * 128],
                                 q_sb[:, ch * 512:(ch + 1) * 512],
                                 start=True, stop=True)
            at = attnp.tile([128, 1024], fATT, name="at", tag="at")
            nc.scalar.activation(at[:, :], et[:, :],
                                 mybir.ActivationFunctionType.Exp,
                                 scale=0.125)
            for ch in range(2):
                nc.tensor.matmul(out_ps[:, ch * 512:(ch + 1) * 512],
                                 vT[:, t * 65:(t + 1) * 65],
                                 at[:, ch * 512:(ch + 1) * 512],
                                 start=(t == 0), stop=(t == 7))

    # ---- phase 3: normalize by softmax sums (row 64 of out_ps) ----
    den_sb = small.tile([1, 1024], fATT, name="den_sb")
    nc.scalar.copy(den_sb[:, :], out_ps[64:65, :])
    ones1 = small.tile([1, 64], fATT, name="ones1")
    nc.vector.memset(ones1[:, :].bitcast(f32), 1.0)
    rb_ps = psA.tile([64, 1024], f32, name="rb_ps", tag="psa")
    for ch in range(2):
        nc.tensor.matmul(rb_ps[:, ch * 512:(ch + 1) * 512], ones1[:, :],
                         den_sb[:, ch * 512:(ch + 1) * 512],
                         start=True, stop=True)
    rb_sb = small.tile([64, 1024], f32, name="rb_sb")
    nc.vector.reciprocal(rb_sb[:, :], rb_ps[:, :])
    osn = small.tile([64, 1024], f16, name="osn")
    nc.vector.tensor_tensor(osn[:, :], out_ps[0:64, :], rb_sb[:, :],
                            mybir.AluOpType.mult)

    # ---- phase 4: pack os -> (s,c) layout, upsample+add, store ----
    os2 = small.tile([128, 512], f16, name="os2")
    for s in range(2):
        nc.scalar.dma_start(os2[s * 64:(s + 1) * 64, :],
                            osn[0:64, s * 512:(s + 1) * 512])

    for g in range(NG):
        ob = obuf.tile([128, GBLK], f16, name="ob", tag="ob")
        for j in range(G):
            hpp = g * G + j
            xs = x_sb[:, hpp * BLK:(hpp + 1) * BLK]
            up = bass.AP(os2.tensor, os2.offset + hpp * 32,
                         [list(os2.ap[0]), [0, DS], [1, 32], [0, DS]])
            nc.vector.tensor_tensor(
                ob[:, j * BLK:(j + 1) * BLK].rearrange(
                    "p (r wp i) -> p r wp i", r=DS, wp=32, i=DS),
                xs.rearrange("p (r wp i) -> p r wp i", r=DS, wp=32, i=DS),
                up, mybir.AluOpType.add)
        for s in range(2):
            nc.sync.dma_start(_group_ap(out_d, g, s),
                              ob[s * 64:(s + 1) * 64, :])


def _build(dup=1):
    nc = bacc.Bacc("TRN2", target_bir_lowering=False, debug=False,
                   num_devices=8)

    x_d = nc.dram_tensor("x", [C, HW], f16, kind="ExternalInput")
    diff_d = nc.dram_tensor("diff", [C, HW], f8, kind="ExternalInput")
    wq_d = nc.dram_tensor("wq", [65, 64], f32, kind="ExternalInput")
    wk_d = nc.dram_tensor("wk", [65, 64], f32, kind="ExternalInput")
    wv_d = nc.dram_tensor("wv", [65, 64], f32, kind="ExternalInput")
    out_d = nc.dram_tensor("out", [C, HW], f16, kind="ExternalOutput")
    drams = (x_d, diff_d, wq_d, wk_d, wv_d, out_d)

    with TileContext(nc) as tc:
        with tc.tile_pool(name="big", bufs=1) as big, \
             tc.tile_pool(name="dstream", bufs=2) as dstream, \
             tc.tile_pool(name="obuf", bufs=2) as obuf, \
             tc.tile_pool(name="small", bufs=1) as small, \
             tc.tile_pool(name="attn", bufs=2) as attnp, \
             tc.tile_pool(name="psA", bufs=1, space="PSUM") as psA, \
             tc.tile_pool(name="psE", bufs=2, space="PSUM") as psE, \
             tc.tile_pool(name="psO", bufs=1, space="PSUM") as psO:
            pools = (big, dstream, obuf, small, attnp, psA, psE, psO)
            for rep in range(dup):
                if rep:
                    tc.strict_bb_all_engine_barrier()
                _emit(nc, tc, pools, drams)

    nc.compile()
    return nc


def make_in_maps(inputs):
    f16np = mybir.dt.np(f16)
    f8np = mybir.dt.np(f8)
    x = np.asarray(inputs["x"], dtype=np.float32).reshape(B, C, HW)
    diff = np.asarray(inputs["diff"], dtype=np.float32).reshape(B, C, HW)
    x = np.ascontiguousarray(x.astype(f16np))
    diff = np.ascontiguousarray(diff.astype(f8np))
    # fold avg-pool 1/64 into the weights; append bias row (K=65 aug)
    inv = 1.0 / (DS * DS)
    wq_aug = np.concatenate(
        [np.asarray(inputs["Wq"]).T * inv,
         np.asarray(inputs["bq"])[None, :]], 0)
    wk_aug = np.concatenate(
        [np.asarray(inputs["Wk"]).T * inv,
         np.asarray(inputs["bk"])[None, :]], 0)
    wv_aug = np.concatenate(
        [np.asarray(inputs["Wv"]).T * inv,
         np.asarray(inputs["bv"])[None, :]], 0)
    wq_aug = np.ascontiguousarray(wq_aug, dtype=np.float32)
    wk_aug = np.ascontiguousarray(wk_aug, dtype=np.float32)
    wv_aug = np.ascontiguousarray(wv_aug, dtype=np.float32)
    return [
        {
            "x": x[b],
            "diff": diff[b],
            "wq": wq_aug, "wk": wk_aug, "wv": wv_aug,
        }
        for b in range(B)
    ]


def kernel(x, diff, Wq, bq, Wk, bk, Wv, bv):
    if "nc" not in _cache:
        _cache["nc"] = _build()
    nc = _cache["nc"]

    in_maps = make_in_maps(dict(x=x, diff=diff, Wq=Wq, bq=bq, Wk=Wk, bk=bk,
                                Wv=Wv, bv=bv))
    res = run_bass_kernel_spmd(nc, in_maps, list(range(B)))
    out = np.stack([np.asarray(res.results[b]["out"], dtype=np.float32)
                    .reshape(C, H, W) for b in range(B)])
    return out


if __name__ == "__main__":
    rng = np.random.default_rng(0)
    xs = rng.standard_normal((B, C, H, W), dtype=np.float32)
    ds = rng.standard_normal((B, C, H, W), dtype=np.float32)
    sc = 1.0 / np.sqrt(C)
    args = dict(
        x=xs, diff=ds,
        Wq=rng.standard_normal((C, C), dtype=np.float32) * sc,
        bq=rng.standard_normal(C, dtype=np.float32) * 0.01,
        Wk=rng.standard_normal((C, C), dtype=np.float32) * sc,
        bk=rng.standard_normal(C, dtype=np.float32) * 0.01,
        Wv=rng.standard_normal((C, C), dtype=np.float32) * sc,
        bv=rng.standard_normal(C, dtype=np.float32) * 0.01,
    )
    out = kernel(**args)
    print("kernel ran, out shape", out.shape, out.dtype)
